# revision 7
# baseline (speedup 1.0000x reference)
"""Trainium2 Bass kernel for nn_LogisticModel.

Computes, for each batch row b:
    logp[b] = sum_t Normal(x_t - 0.9*x_{t-1} - sigmoid(s_t), 0.1).logpdf(0)
            = -0.5/0.01 * sum_t resid_t^2 + T * (-ln(0.1) - 0.5*ln(2*pi))
with x_{-1} = 0.  Pure elementwise + row reduction; sharded by batch rows
across 8 NeuronCores (512 rows per core).

DMA-drain-bound.  SDMA engine 15 (SBUF partitions 120-127) runs ~20% slower
than the other engines (it co-hosts the HWDGE queue bookkeeping), so rows
120-127 of each 128-row group take only cols [0, 6144) through partitions
120-127; their cols [6144, 8192) are relocated as 16 uniform [8, 512]
blocks, one per partition octet, loaded by all 16 engines and computed as
single full-partition ops.  The relocated partials are merged into logp via
tiny SBUF->SBUF fixup DMAs issued from the idle gpsimd engine (compute ops
must start at a multiple-of-32 partition; DMA has no such restriction).
The last chunk of the last group is narrow (512 cols) to keep the post-DMA
compute tail short.

Self-contained: hardcodes B=4096, T=8192, fp32.
"""

import math
import os
import sys

import numpy as np

sys.path.insert(0, "/opt/trn_rl_repo")

import concourse.bacc as bacc  # noqa: E402
import concourse.bass as bass  # noqa: E402
import concourse.tile as tile  # noqa: E402
from concourse import mybir  # noqa: E402
from concourse.bass_utils import run_bass_kernel_spmd  # noqa: E402

GAIN = 1.0
DECAY = 0.9
NOISE = 0.1
LOG_2PI = math.log(2.0 * math.pi)

B, T = 4096, 8192
N_CORES = 8
ROWS_PER_CORE = B // N_CORES          # 512
P = 128                               # SBUF partitions
N_GROUP = ROWS_PER_CORE // P          # 4 row-groups per core

SPLIT_COL = 6144                      # rows 120-127: cols [0, SPLIT_COL) main
RBLK = 512                            # relocated block width
N_RBLK = (T - SPLIT_COL) // RBLK      # 4 blocks per group, 16 octets total

C1 = -0.5 / (NOISE * NOISE)                      # -50.0
C2 = T * (-math.log(NOISE) - 0.5 * LOG_2PI)      # per-row additive constant

_cache = {}


def _build(bufs=5):
    """Build and schedule the per-core Tile kernel (same program on all 8)."""
    nc = bacc.Bacc("TRN2", target_bir_lowering=False, debug=False,
                   num_devices=N_CORES)
    f32 = mybir.dt.float32
    s_d = nc.dram_tensor("s", [ROWS_PER_CORE, T], f32, kind="ExternalInput").ap()
    x_d = nc.dram_tensor("x", [ROWS_PER_CORE, T], f32, kind="ExternalInput").ap()
    o_d = nc.dram_tensor("o", [P, N_GROUP], f32, kind="ExternalOutput").ap()

    Alu = mybir.AluOpType
    Act = mybir.ActivationFunctionType

    # Main-stream chunk widths per group: (width, rows 120-127 included?).
    # Cols >= SPLIT_COL run on partitions 0-119 only.  The last chunk of the
    # last group is narrow so the post-last-DMA compute chain is short.
    full_plan = [(2048, True)] * 3
    chunk_plan = [full_plan + [(2048, False)]] * (N_GROUP - 1) + [
        full_plan + [(1536, False), (512, False)]]
    n_chunks = [len(ws) for ws in chunk_plan]
    acc_cols = sum(n_chunks)

    with tile.TileContext(nc) as tc:
        with (
            tc.tile_pool(name="io", bufs=bufs) as io,
            tc.tile_pool(name="rel", bufs=1) as rel,
            tc.tile_pool(name="accp", bufs=1) as accp,
        ):
            acc = accp.tile([P, acc_cols], f32)   # per-chunk partial sums
            accr = accp.tile([P, 1], f32)         # relocated partial sums
            fix2 = accp.tile([P, 16], f32)        # gathered reloc partials
            fix = accp.tile([P, N_GROUP], f32)
            logp = accp.tile([P, N_GROUP], f32)

            nc.vector.memset(acc[:], 0.0)
            nc.vector.memset(fix2[:], 0.0)

            # Relocated blocks: (group g, block q) -> partition octet
            # 8*(4g + q).  Uniform [8, RBLK] shape, all octets filled, so the
            # whole tile is computed with 4 full-partition ops.
            s_r = rel.tile([P, RBLK], f32, tag="sr")
            x_r = rel.tile([P, RBLK + 1], f32, tag="xr")
            c_r = rel.tile([P, RBLK], f32, tag="cr")
            u_r = rel.tile([P, RBLK], f32, tag="ur")

            def issue_reloc_load(o):
                g, q = divmod(o, N_RBLK)
                r0 = g * P + 120
                po = 8 * o
                lo = SPLIT_COL + q * RBLK
                nc.sync.dma_start(out=s_r[po:po + 8, :],
                                  in_=s_d[r0:r0 + 8, lo:lo + RBLK])
                nc.sync.dma_start(out=x_r[po:po + 8, :],
                                  in_=x_d[r0:r0 + 8, lo - 1:lo + RBLK])

            def reloc_compute():
                # one set of full-partition ops over all 16 blocks
                nc.scalar.activation(out=c_r[:], in_=s_r[:], func=Act.Sigmoid,
                                     scale=GAIN)
                nc.vector.tensor_sub(u_r[:], x_r[:, 1:RBLK + 1], c_r[:])
                nc.vector.scalar_tensor_tensor(
                    out=c_r[:], in0=x_r[:, 0:RBLK], scalar=DECAY, in1=u_r[:],
                    op0=Alu.mult, op1=Alu.subtract,
                )
                nc.scalar.activation(out=u_r[:], in_=c_r[:], func=Act.Square,
                                     accum_out=accr[:])
                # gather each octet's partial onto partitions 120-127 (idle
                # gpsimd engine; DMA has no partition-base restriction)
                for o in range(16):
                    nc.gpsimd.dma_start(out=fix2[120:128, o:o + 1],
                                        in_=accr[8 * o:8 * o + 8, 0:1])

            # --- main stream.  Reloc loads are trickled in (2 blocks per
            # chunk) during groups 0-1 so they neither starve the engine
            # rings nor delay later main loads; the reloc compute + fixup
            # runs mid-stream, off the critical tail.
            reloc_issued = 0
            it = 0
            for g in range(N_GROUP):
                r0 = g * P
                col = 0
                for j, (w, fullp) in enumerate(chunk_plan[g]):
                    lo, hi = col, col + w
                    np_ = P if fullp else 120
                    s_t = io.tile([P, w], f32, tag="s")
                    xx = io.tile([P, w + 1], f32, tag="xx")
                    c_t = io.tile([P, w], f32, tag="c")
                    u_t = io.tile([P, w], f32, tag="u")

                    nc.sync.dma_start(out=s_t[0:np_, :],
                                      in_=s_d[r0:r0 + np_, lo:hi])
                    if j == 0:
                        nc.vector.memset(xx[:, 0:1], 0.0)
                        nc.sync.dma_start(out=xx[0:np_, 1:w + 1],
                                          in_=x_d[r0:r0 + np_, 0:w])
                    else:
                        # one-column overlap supplies x_{t-1} across the seam
                        nc.sync.dma_start(out=xx[0:np_, 0:w + 1],
                                          in_=x_d[r0:r0 + np_, lo - 1:hi])

                    if reloc_issued < 16:
                        issue_reloc_load(reloc_issued)
                        issue_reloc_load(reloc_issued + 1)
                        reloc_issued += 2

                    # bias = sigmoid(GAIN * s)
                    nc.scalar.activation(out=c_t[0:np_, :], in_=s_t[0:np_, :],
                                         func=Act.Sigmoid, scale=GAIN)
                    # u = x - bias (even chunks on the otherwise idle gpsimd
                    # engine: the two DVE ops alone exceed the per-chunk DMA
                    # drain time, so DVE would gate buffer recycling)
                    sub_eng = nc.gpsimd if it % 2 == 0 else nc.vector
                    sub_eng.tensor_sub(u_t[0:np_, :], xx[0:np_, 1:w + 1],
                                       c_t[0:np_, :])
                    # -resid = (0.9 * x_prev) - u ; sign dies in Square
                    nc.vector.scalar_tensor_tensor(
                        out=c_t[0:np_, :], in0=xx[0:np_, 0:w], scalar=DECAY,
                        in1=u_t[0:np_, :], op0=Alu.mult, op1=Alu.subtract,
                    )
                    # acc[:, it] = sum_t resid^2
                    nc.scalar.activation(out=u_t[0:np_, :], in_=c_t[0:np_, :],
                                         func=Act.Square,
                                         accum_out=acc[0:np_, it:it + 1])
                    col += w
                    it += 1
                if g == 1:
                    reloc_compute()
                if g == 2:
                    nc.vector.tensor_reduce(
                        out=fix[:],
                        in_=fix2[:].rearrange("p (g q) -> p g q", q=N_RBLK),
                        axis=mybir.AxisListType.X, op=Alu.add)

            # group sums over each group's partials, then merge the
            # relocated partials and apply the affine transform.
            base = 0
            for g in range(N_GROUP):
                nc.vector.tensor_reduce(
                    out=logp[:, g:g + 1], in_=acc[:, base:base + n_chunks[g]],
                    axis=mybir.AxisListType.X, op=Alu.add)
                base += n_chunks[g]
            nc.vector.tensor_add(logp[:], logp[:], fix[:])
            nc.vector.tensor_scalar(
                out=logp[:], in0=logp[:], scalar1=C1, scalar2=C2,
                op0=Alu.mult, op1=Alu.add,
            )
            nc.sync.dma_start(out=o_d[:], in_=logp[:])

    nc.compile()
    return nc


def _run(s, x, trace=False, **build_kwargs):
    key = tuple(sorted(build_kwargs.items()))
    if key not in _cache:
        _cache[key] = _build(**build_kwargs)
    nc = _cache[key]

    in_maps = []
    for k in range(N_CORES):
        r0 = k * ROWS_PER_CORE
        in_maps.append({
            "s": np.ascontiguousarray(s[r0:r0 + ROWS_PER_CORE]),
            "x": np.ascontiguousarray(x[r0:r0 + ROWS_PER_CORE]),
        })

    res = run_bass_kernel_spmd(nc, in_maps, list(range(N_CORES)), trace=trace)

    out = np.empty((B,), dtype=np.float32)
    for k in range(N_CORES):
        # o[p, g] holds the row g*P + p of this core's shard
        out[k * ROWS_PER_CORE:(k + 1) * ROWS_PER_CORE] = (
            np.asarray(res.results[k]["o"]).T.reshape(-1)
        )
    return out, res


def kernel(s, x):
    out, _ = _run(np.asarray(s, dtype=np.float32), np.asarray(x, dtype=np.float32))
    return out


if __name__ == "__main__":
    rng = np.random.default_rng(0)
    s = rng.standard_normal((B, T), dtype=np.float32)
    x = rng.standard_normal((B, T), dtype=np.float32)
    out = kernel(s, x)
    print(out.shape, out.dtype, out[:4])


# revision 13
# speedup vs baseline: 1.2151x; 1.2151x over previous
"""Trainium2 Bass kernel for nn_LogisticModel.

Computes, for each batch row b:
    logp[b] = sum_t Normal(x_t - 0.9*x_{t-1} - sigmoid(s_t), 0.1).logpdf(0)
            = -0.5/0.01 * sum_t resid_t^2 + T * (-ln(0.1) - 0.5*ln(2*pi))
with x_{-1} = 0.  Pure elementwise + row reduction; sharded by batch rows
across 8 NeuronCores (512 rows per core).

DMA-drain-bound.  SDMA engine 15 (SBUF partitions 120-127) runs ~20% slower
than the other engines (it co-hosts the HWDGE queue bookkeeping), so rows
120-127 of each 128-row group take only cols [0, 6144) through partitions
120-127; their cols [6144, 8192) are relocated as 16 uniform [8, 512]
blocks, one per partition octet (row 120+i block q -> partition 32g+8q+i),
computed as full-partition ops mid-stream.  The relocated partials are
gathered onto partitions 120-127 by tiny SBUF->SBUF DMAs (DMA has no
partition-base restriction; compute ops must start at a multiple-of-32
partition) and merged into logp.  The last chunk of the last group is
narrow (512 cols) to keep the post-DMA compute tail short.

Self-contained: hardcodes B=4096, T=8192, fp32.
"""

import math
import os
import sys

import numpy as np

sys.path.insert(0, "/opt/trn_rl_repo")

import concourse.bacc as bacc  # noqa: E402
import concourse.bass as bass  # noqa: E402
import concourse.tile as tile  # noqa: E402
from concourse import mybir  # noqa: E402
from concourse.bass_utils import run_bass_kernel_spmd  # noqa: E402

GAIN = 1.0
DECAY = 0.9
NOISE = 0.1
LOG_2PI = math.log(2.0 * math.pi)

B, T = 4096, 8192
N_CORES = 8
ROWS_PER_CORE = B // N_CORES          # 512
P = 128                               # SBUF partitions
N_GROUP = ROWS_PER_CORE // P          # 4 row-groups per core

SPLIT_COL = 6144                      # rows 120-127: cols [0, SPLIT_COL) main
RBLK = 512                            # relocated block width
N_RBLK = (T - SPLIT_COL) // RBLK      # 4 blocks per group

C1 = -0.5 / (NOISE * NOISE)                      # -50.0
C2 = T * (-math.log(NOISE) - 0.5 * LOG_2PI)      # per-row additive constant

_cache = {}


def _build(bufs=5):
    """Build and schedule the per-core Tile kernel (same program on all 8)."""
    nc = bacc.Bacc("TRN2", target_bir_lowering=False, debug=False,
                   num_devices=N_CORES)
    f32 = mybir.dt.float32
    s_d = nc.dram_tensor("s", [ROWS_PER_CORE, T], f32, kind="ExternalInput").ap()
    x_d = nc.dram_tensor("x", [ROWS_PER_CORE, T], f32, kind="ExternalInput").ap()
    o_d = nc.dram_tensor("o", [P, N_GROUP], f32, kind="ExternalOutput").ap()

    Alu = mybir.AluOpType
    Act = mybir.ActivationFunctionType

    # Main-stream chunk plan per group: (width, n_partitions).  Cols >=
    # SPLIT_COL run on partitions 0-119 only.  The last chunk of the last
    # group is narrow so the post-last-DMA compute chain is short.
    full3 = [(2048, P)] * 3
    chunk_plan = [full3 + [(2048, 120)]] * (N_GROUP - 1) + [
        full3 + [(1536, 120), (512, 120)]]
    n_chunks = [len(ws) for ws in chunk_plan]
    acc_cols = sum(n_chunks)

    with tile.TileContext(nc) as tc:
        with (
            tc.tile_pool(name="io", bufs=bufs) as io,
            tc.tile_pool(name="rel", bufs=1) as rel,
            tc.tile_pool(name="accp", bufs=1) as accp,
        ):
            acc = accp.tile([P, acc_cols], f32)   # per-chunk partial sums
            accr = accp.tile([P, 1], f32)         # relocated partial sums
            fix2 = accp.tile([P, N_GROUP * N_RBLK], f32)
            fix = accp.tile([P, N_GROUP], f32)
            logp = accp.tile([P, N_GROUP], f32)

            nc.vector.memset(acc[:], 0.0)
            nc.vector.memset(fix2[:], 0.0)

            # Relocated data: (group g, block q) -> partition octet
            # 8*(4g + q), i.e. row 120+i block q at partition 32g+8q+i;
            # every SDMA engine serves exactly one octet.
            s_r = rel.tile([P, RBLK], f32, tag="sr")
            x_r = rel.tile([P, RBLK + 1], f32, tag="xr")
            c_r = rel.tile([P, RBLK], f32, tag="cr")
            u_r = rel.tile([P, RBLK], f32, tag="ur")

            def issue_reloc_load(o):
                g, q = divmod(o, N_RBLK)
                r0, po = g * P + 120, 8 * o
                lo = SPLIT_COL + q * RBLK
                nc.sync.dma_start(out=s_r[po:po + 8, :],
                                  in_=s_d[r0:r0 + 8, lo:lo + RBLK])
                # one-column overlap supplies the block's x_{t-1} seam
                nc.sync.dma_start(out=x_r[po:po + 8, 0:RBLK + 1],
                                  in_=x_d[r0:r0 + 8, lo - 1:lo + RBLK])

            def reloc_compute():
                nc.scalar.activation(out=c_r[:], in_=s_r[:], func=Act.Sigmoid,
                                     scale=GAIN)
                nc.vector.tensor_sub(u_r[:], x_r[:, 1:RBLK + 1], c_r[:])
                nc.vector.scalar_tensor_tensor(
                    out=c_r[:], in0=x_r[:, 0:RBLK], scalar=DECAY, in1=u_r[:],
                    op0=Alu.mult, op1=Alu.subtract,
                )
                nc.scalar.activation(out=u_r[:], in_=c_r[:], func=Act.Square,
                                     accum_out=accr[:])

            # --- main stream.  Reloc loads trickle in (2 blocks per chunk)
            # during groups 0-1 so they neither starve the engine rings nor
            # delay later main loads.
            reloc_issued = 0
            it = 0
            for g in range(N_GROUP):
                r0 = g * P
                col = 0
                for j, (w, np_) in enumerate(chunk_plan[g]):
                    lo, hi = col, col + w
                    s_t = io.tile([P, w], f32, tag="s")
                    xx = io.tile([P, w + 1], f32, tag="xx")
                    c_t = io.tile([P, w], f32, tag="c")
                    u_t = io.tile([P, w], f32, tag="u")

                    nc.sync.dma_start(out=s_t[0:np_, :],
                                      in_=s_d[r0:r0 + np_, lo:hi])
                    if j == 0:
                        nc.vector.memset(xx[:, 0:1], 0.0)
                        nc.sync.dma_start(out=xx[0:np_, 1:w + 1],
                                          in_=x_d[r0:r0 + np_, 0:w])
                    else:
                        # one-column overlap supplies x_{t-1} across the seam
                        nc.sync.dma_start(out=xx[0:np_, 0:w + 1],
                                          in_=x_d[r0:r0 + np_, lo - 1:hi])
                    if reloc_issued < N_GROUP * N_RBLK:
                        issue_reloc_load(reloc_issued)
                        issue_reloc_load(reloc_issued + 1)
                        reloc_issued += 2

                    # bias = sigmoid(GAIN * s)
                    nc.scalar.activation(out=c_t[0:np_, :], in_=s_t[0:np_, :],
                                         func=Act.Sigmoid, scale=GAIN)
                    # u = x - bias
                    nc.vector.tensor_sub(u_t[0:np_, :], xx[0:np_, 1:w + 1],
                                         c_t[0:np_, :])
                    # -resid = (0.9 * x_prev) - u ; sign dies in Square
                    nc.vector.scalar_tensor_tensor(
                        out=c_t[0:np_, :], in0=xx[0:np_, 0:w], scalar=DECAY,
                        in1=u_t[0:np_, :], op0=Alu.mult, op1=Alu.subtract,
                    )
                    # acc[:, it] = sum_t resid^2
                    nc.scalar.activation(out=u_t[0:np_, :], in_=c_t[0:np_, :],
                                         func=Act.Square,
                                         accum_out=acc[0:np_, it:it + 1])
                    col += w
                    it += 1
                if g == 1:
                    reloc_compute()

            # Fixup: gather each octet's relocated partials onto partitions
            # 120-127 (sync engine is idle once the main loads are issued;
            # DMA has no partition-base restriction).
            for o in range(N_GROUP * N_RBLK):
                nc.sync.dma_start(out=fix2[120:128, o:o + 1],
                                  in_=accr[8 * o:8 * o + 8, 0:1])

            # tail: block-reduce the fixups, per-group sums, merge, affine
            nc.vector.tensor_reduce(
                out=fix[:], in_=fix2[:].rearrange("p (g q) -> p g q",
                                                  q=N_RBLK),
                axis=mybir.AxisListType.X, op=Alu.add)
            base = 0
            for g in range(N_GROUP):
                nc.vector.tensor_reduce(
                    out=logp[:, g:g + 1], in_=acc[:, base:base + n_chunks[g]],
                    axis=mybir.AxisListType.X, op=Alu.add)
                base += n_chunks[g]
            nc.vector.tensor_add(logp[:], logp[:], fix[:])
            nc.vector.tensor_scalar(
                out=logp[:], in0=logp[:], scalar1=C1, scalar2=C2,
                op0=Alu.mult, op1=Alu.add,
            )
            nc.sync.dma_start(out=o_d[:], in_=logp[:])

    nc.compile()
    return nc


def _run(s, x, trace=False, **build_kwargs):
    key = tuple(sorted(build_kwargs.items()))
    if key not in _cache:
        _cache[key] = _build(**build_kwargs)
    nc = _cache[key]

    in_maps = []
    for k in range(N_CORES):
        r0 = k * ROWS_PER_CORE
        in_maps.append({
            "s": np.ascontiguousarray(s[r0:r0 + ROWS_PER_CORE]),
            "x": np.ascontiguousarray(x[r0:r0 + ROWS_PER_CORE]),
        })

    res = run_bass_kernel_spmd(nc, in_maps, list(range(N_CORES)), trace=trace)

    out = np.empty((B,), dtype=np.float32)
    for k in range(N_CORES):
        # o[p, g] holds the row g*P + p of this core's shard
        out[k * ROWS_PER_CORE:(k + 1) * ROWS_PER_CORE] = (
            np.asarray(res.results[k]["o"]).T.reshape(-1)
        )
    return out, res


def kernel(s, x):
    out, _ = _run(np.asarray(s, dtype=np.float32), np.asarray(x, dtype=np.float32))
    return out


if __name__ == "__main__":
    rng = np.random.default_rng(0)
    s = rng.standard_normal((B, T), dtype=np.float32)
    x = rng.standard_normal((B, T), dtype=np.float32)
    out = kernel(s, x)
    print(out.shape, out.dtype, out[:4])
